# revision 6
# baseline (speedup 1.0000x reference)
"""Deformable causal conv1d Trainium2 kernel (v8).

vs v7: sync-overhead-focused rework.
  * Pair-batched gathers: ONE dma_start loads both groups of a pair
    (X: [128, 2*XW], dx: [128, 1024]) -> half the DMA issues + sems.
  * Pair-batched DVE chain: P and S are single [128,1024] TTs; dd is
    two 512-col activations into one shared tile (bias differs per g).
  * Raw matmuls back to full-128 contraction (v6b style, no
    tile_position pairing -- that raced on HW).
  * Weights preloaded as two flat DRAM images ([128, NG*128] raw,
    [128, NG*512] main); rw is loaded FIRST so the PE prologue is
    short (v7 lost 41us waiting for rw behind 4MB of wm loads).
  * Aligned gathers + host-precomputed dx (v7's win, kept): X1 at
    col 0 -> all TTs run in DVE 2x mode.

Sharding: 8 cores = 4 batches x 2 time-halves. No collectives.
"""

import numpy as np
import ml_dtypes
import bass_rust

import concourse.bass as bass
import concourse.tile as tile
from concourse import bacc, mybir

F32 = mybir.dt.float32
BF16 = mybir.dt.bfloat16
Alu = mybir.AluOpType
Act = mybir.ActivationFunctionType

B, C, T = 4, 512, 4096
K, OK = 8, 3
O = 512  # C_out
H = 16  # left halo columns in the x slice
HR = 8  # right pad columns
TH = 2048  # time columns per core
N_CORES = 8
NG = 32  # channel groups of 16
XW = 520  # X gather width per group: X1 at col t, raw rhs at col t+5 / t+1
DXW = 512


def build_device_program(th=TH, tt=512):
    n_chunks = th // tt
    xrow = H + th + HR  # 2072
    npair = NG // 2

    nc = bacc.Bacc("TRN2", target_bir_lowering=False, debug=False)

    x_d = nc.dram_tensor("xcore", [C, xrow], BF16, kind="ExternalInput")
    dx_d = nc.dram_tensor("dxcore", [C, xrow], BF16, kind="ExternalInput")
    wm_d = nc.dram_tensor("wmain", [128, NG * O], BF16, kind="ExternalInput").ap()
    rw_d = nc.dram_tensor("raww", [128, NG * 128], BF16, kind="ExternalInput").ap()
    offb_d = nc.dram_tensor("offb", [128, NG], F32, kind="ExternalInput").ap()
    bias_d = nc.dram_tensor("biasr", [128, O // 128], F32, kind="ExternalInput").ap()
    out_d = nc.dram_tensor("out", [O, th], BF16, kind="ExternalOutput").ap()

    n_ot = O // 128

    def gather_src(dram, g, t0, width):
        """DRAM source AP (k:8, c:16, col:width);
        element = arr[g*16+c, H + t0 - 7 + k + col]."""
        a = dram.ap()
        a.ap = bass_rust.VecI64Pair([(1, K), (xrow, 16), (1, width)])
        a.offset = (g * 16) * xrow + (H + t0 - 7)
        return a

    with tile.TileContext(nc) as tc:
        with (
            tc.tile_pool(name="const", bufs=1) as cpool,
            tc.tile_pool(name="xb", bufs=6) as xbpool,
            tc.tile_pool(name="dxb", bufs=6) as dxbpool,
            tc.tile_pool(name="chain", bufs=4) as chain,
            tc.tile_pool(name="spool", bufs=6) as spool,
            tc.tile_pool(name="outp", bufs=2) as outp,
            tc.tile_pool(name="psum", bufs=1, space="PSUM") as pspool,
            tc.tile_pool(name="rawps", bufs=2, space="PSUM") as rawps,
        ):
            # ---- resident constants; rw FIRST so raw matmuls start early ----
            offb_sb = cpool.tile([128, NG], F32, tag="offb")
            nc.sync.dma_start(offb_sb[:], offb_d)
            bias_sb = cpool.tile([128, n_ot], F32, tag="biasr")
            nc.sync.dma_start(bias_sb[:], bias_d)
            rw_sb = cpool.tile([128, NG * 128], BF16, tag="rwall")
            nc.sync.dma_start(rw_sb[:], rw_d)
            wm_sb = cpool.tile([128, NG * O], BF16, tag="wmall")

            def emit_evac(ps_prev, t0_prev):
                for ot in range(n_ot):
                    out_sb = outp.tile([128, tt], BF16, tag="osb", name="out_sb")
                    nc.scalar.activation(
                        out_sb[:], ps_prev[ot][:], Act.Identity,
                        bias=bias_sb[:, ot : ot + 1],
                    )
                    nc.sync.dma_start(
                        out_d[ot * 128 : (ot + 1) * 128, t0_prev : t0_prev + tt],
                        out_sb[:],
                    )

            wm_loaded = False
            prev = None
            for chunk in range(n_chunks):
                t0 = chunk * tt
                ps = {}
                for ot in range(n_ot):
                    ps[ot] = pspool.tile([128, tt], F32, tag=f"ps{ot}", name=f"ps{ot}")

                for gp in range(npair):
                    xp = xbpool.tile([128, 2 * XW], BF16, tag="Xp")
                    dxp = dxbpool.tile([128, 2 * DXW], BF16, tag="DXp")
                    for g2 in range(2):
                        g = 2 * gp + g2
                        nc.gpsimd.dma_start(
                            xp[:, g2 * XW : (g2 + 1) * XW],
                            gather_src(x_d, g, t0, XW),
                        )
                        nc.sync.dma_start(
                            dxp[:, g2 * DXW : (g2 + 1) * DXW],
                            gather_src(dx_d, g, t0, DXW),
                        )
                    if not wm_loaded:
                        # main weights after the first pair's gathers are queued
                        for i in range(4):
                            nc.sync.dma_start(
                                wm_sb[:, i * 8 * O : (i + 1) * 8 * O],
                                wm_d[:, i * 8 * O : (i + 1) * 8 * O],
                            )
                        wm_loaded = True

                    rp = rawps.tile([128, 2 * tt], F32, tag="rawps", name=f"rp{chunk}_{gp}")
                    ddp = chain.tile([128, 2 * tt], BF16, tag="dd")
                    for g2 in range(2):
                        g = 2 * gp + g2
                        # raw offset-conv matmul: full-128 contraction, banded
                        # weights; rhs col base 5 (even g, taps k'=0..2) or
                        # 1 (odd g, taps k'=4..6) absorbs the tap base.
                        rbase = g2 * XW + (5 if g2 == 0 else 1)
                        nc.tensor.matmul(
                            rp[:, g2 * tt : (g2 + 1) * tt],
                            rw_sb[:, g * 128 : (g + 1) * 128],
                            xp[:, rbase : rbase + tt],
                            start=True, stop=True,
                        )
                        nc.scalar.activation(
                            ddp[:, g2 * tt : (g2 + 1) * tt],
                            rp[:, g2 * tt : (g2 + 1) * tt],
                            Act.Abs, bias=offb_sb[:, g : g + 1],
                        )
                    P = chain.tile([128, 2 * tt], BF16, tag="P")
                    nc.vector.tensor_tensor(P[:], ddp[:], dxp[:], Alu.mult)
                    S = spool.tile([128, 2 * tt], BF16, tag="S")
                    x1 = xp[:]
                    x1.ap = bass_rust.VecI64Pair([(2 * XW, 128), (XW, 2), (1, tt)])
                    x1.offset = 0
                    nc.vector.tensor_tensor(S[:], x1, P[:], Alu.subtract)
                    for g2 in range(2):
                        g = 2 * gp + g2
                        for ot in range(n_ot):
                            nc.tensor.matmul(
                                ps[ot][:],
                                wm_sb[:, g * O + ot * 128 : g * O + (ot + 1) * 128],
                                S[:, g2 * tt : (g2 + 1) * tt],
                                start=(g == 0),
                                stop=(g == NG - 1),
                            )
                    if gp == 0 and prev is not None:
                        emit_evac(*prev)
                        prev = None
                prev = (ps, t0)

            emit_evac(*prev)

    nc.compile()
    return nc


def prep_host_inputs(x, offset_w, offset_b, weight, bias, th=TH):
    ow = offset_w.reshape(C, K, OK).astype(np.float32)  # [c, k, j]
    ob = offset_b.reshape(C, K).astype(np.float32)

    # main weight flat image: wm[p=k*16+cl, g*O+o] = weight[o, g*16+cl, k]
    wm = np.ascontiguousarray(
        weight.transpose(1, 2, 0)  # [C, K, O]
        .reshape(NG, 16, K, O)
        .transpose(2, 1, 0, 3)  # [k, cl, g, o]
        .reshape(128, NG * O)
    ).astype(ml_dtypes.bfloat16)

    # raw weight flat image: per g a [128,128] block; even g rows j*16+cl,
    # odd g rows 64+j*16+cl; cols k*16+cl
    rw = np.zeros((128, NG, 128), np.float32)
    cl = np.arange(16)
    for g in range(NG):
        base = 0 if g % 2 == 0 else 64
        for j in range(OK):
            for k in range(K):
                rw[base + j * 16 + cl, g, k * 16 + cl] = ow[g * 16 + cl, k, j]
    rw = np.ascontiguousarray(rw.reshape(128, NG * 128)).astype(ml_dtypes.bfloat16)

    offb = np.ascontiguousarray(
        ob.reshape(NG, 16, K).transpose(2, 1, 0).reshape(128, NG)
    ).astype(np.float32)
    biasr = np.ascontiguousarray(bias.reshape(O // 128, 128).T).astype(np.float32)

    # dx[b, c, v] = x[b, c, v] - x[b, c, v-1]  (x[-1] == 0)
    dxg = np.diff(np.pad(x, ((0, 0), (0, 0), (1, 0))), axis=2)

    xcores, dxcores = [], []
    n_th = T // th
    for core in range(N_CORES):
        b, thi = divmod(core, n_th)
        t0 = thi * th
        xc = np.zeros((C, H + th + HR), np.float32)
        dxc = np.zeros((C, H + th + HR), np.float32)
        xc[:, H : H + th] = x[b, :, t0 : t0 + th]
        dxc[:, H : H + th] = dxg[b, :, t0 : t0 + th]
        if t0 >= H:
            xc[:, :H] = x[b, :, t0 - H : t0]
            dxc[:, :H] = dxg[b, :, t0 - H : t0]
        xcores.append(np.ascontiguousarray(xc.astype(ml_dtypes.bfloat16)))
        dxcores.append(np.ascontiguousarray(dxc.astype(ml_dtypes.bfloat16)))
    return wm, rw, offb, biasr, xcores, dxcores


_PROGRAM_CACHE = {}


def _get_program():
    key = "main"
    if key not in _PROGRAM_CACHE:
        _PROGRAM_CACHE[key] = build_device_program()
    return _PROGRAM_CACHE[key]


def run_on_hw(inputs, trace=False, **kw):
    from concourse.bass_utils import run_bass_kernel_spmd

    nc = _get_program()
    wm, rw, offb, biasr, xcores, dxcores = prep_host_inputs(
        inputs["x"], inputs["offset_w"], inputs["offset_b"],
        inputs["weight"], inputs["bias"],
    )
    in_maps = [
        {
            "xcore": xcores[core],
            "dxcore": dxcores[core],
            "wmain": wm,
            "raww": rw,
            "offb": offb,
            "biasr": biasr,
        }
        for core in range(N_CORES)
    ]
    res = run_bass_kernel_spmd(
        nc, in_maps, core_ids=list(range(N_CORES)), trace=trace, **kw
    )
    return res


def kernel(**inputs) -> np.ndarray:
    res = run_on_hw(inputs)
    out = np.empty((B, O, T), np.float32)
    n_th = T // TH
    for core in range(N_CORES):
        b, thi = divmod(core, n_th)
        out[b, :, thi * TH : (thi + 1) * TH] = res.results[core]["out"].astype(
            np.float32
        )
    return out


if __name__ == "__main__":
    z = np.load("/root/problem/inputs.npz")
    out = kernel(**{k: z[k] for k in z.files})
    print("kernel out:", out.shape, out.dtype, float(np.abs(out).max()))


# revision 8
# speedup vs baseline: 1.0758x; 1.0758x over previous
"""Deformable causal conv1d Trainium2 kernel (v8).

vs v7: sync-overhead-focused rework.
  * Pair-batched gathers: ONE dma_start loads both groups of a pair
    (X: [128, 2*XW], dx: [128, 1024]) -> half the DMA issues + sems.
  * Pair-batched DVE chain: P and S are single [128,1024] TTs; dd is
    two 512-col activations into one shared tile (bias differs per g).
  * Raw matmuls back to full-128 contraction (v6b style, no
    tile_position pairing -- that raced on HW).
  * Weights preloaded as two flat DRAM images ([128, NG*128] raw,
    [128, NG*512] main); rw is loaded FIRST so the PE prologue is
    short (v7 lost 41us waiting for rw behind 4MB of wm loads).
  * Aligned gathers + host-precomputed dx (v7's win, kept): X1 at
    col 0 -> all TTs run in DVE 2x mode.

Sharding: 8 cores = 4 batches x 2 time-halves. No collectives.
"""

import numpy as np
import ml_dtypes
import bass_rust

import concourse.bass as bass
import concourse.tile as tile
from concourse import bacc, mybir

F32 = mybir.dt.float32
BF16 = mybir.dt.bfloat16
Alu = mybir.AluOpType
Act = mybir.ActivationFunctionType

B, C, T = 4, 512, 4096
K, OK = 8, 3
O = 512  # C_out
H = 16  # left halo columns in the x slice
HR = 8  # right pad columns
TH = 2048  # time columns per core
N_CORES = 8
NG = 32  # channel groups of 16
XW = 520  # X gather width per group: X1 at col t, raw rhs at col t+5 / t+1
DXW = 512


def build_device_program(th=TH, tt=512):
    n_chunks = th // tt
    xrow = H + th + HR  # 2072
    npair = NG // 2

    nc = bacc.Bacc("TRN2", target_bir_lowering=False, debug=False)

    x_d = nc.dram_tensor("xcore", [C, xrow], BF16, kind="ExternalInput")
    dx_d = nc.dram_tensor("dxcore", [C, xrow], BF16, kind="ExternalInput")
    wm_d = nc.dram_tensor("wmain", [128, NG * O], BF16, kind="ExternalInput").ap()
    rw_d = nc.dram_tensor("raww", [128, NG * 128], BF16, kind="ExternalInput").ap()
    offb_d = nc.dram_tensor("offb", [128, NG], F32, kind="ExternalInput").ap()
    bias_d = nc.dram_tensor("biasr", [128, O // 128], F32, kind="ExternalInput").ap()
    out_d = nc.dram_tensor("out", [O, th], BF16, kind="ExternalOutput").ap()

    n_ot = O // 128

    def gather_src(dram, g, t0, width):
        """DRAM source AP (k:8, c:16, col:width);
        element = arr[g*16+c, H + t0 - 7 + k + col]."""
        a = dram.ap()
        a.ap = bass_rust.VecI64Pair([(1, K), (xrow, 16), (1, width)])
        a.offset = (g * 16) * xrow + (H + t0 - 7)
        return a

    with tile.TileContext(nc) as tc:
        with (
            tc.tile_pool(name="const", bufs=1) as cpool,
            tc.tile_pool(name="xb", bufs=6) as xbpool,
            tc.tile_pool(name="dxb", bufs=6) as dxbpool,
            tc.tile_pool(name="chain", bufs=4) as chain,
            tc.tile_pool(name="spool", bufs=6) as spool,
            tc.tile_pool(name="outp", bufs=2) as outp,
            tc.tile_pool(name="psum", bufs=1, space="PSUM") as pspool,
            tc.tile_pool(name="rawps", bufs=2, space="PSUM") as rawps,
        ):
            # ---- resident constants; rw FIRST so raw matmuls start early ----
            offb_sb = cpool.tile([128, NG], F32, tag="offb")
            nc.sync.dma_start(offb_sb[:], offb_d)
            bias_sb = cpool.tile([128, n_ot], F32, tag="biasr")
            nc.sync.dma_start(bias_sb[:], bias_d)
            rw_sb = cpool.tile([128, NG * 128], BF16, tag="rwall")
            nc.sync.dma_start(rw_sb[:], rw_d)
            wm_sb = cpool.tile([128, NG * O], BF16, tag="wmall")

            def emit_evac(ps_prev, t0_prev):
                for ot in range(n_ot):
                    out_sb = outp.tile([128, tt], BF16, tag="osb", name="out_sb")
                    nc.scalar.activation(
                        out_sb[:], ps_prev[ot][:], Act.Identity,
                        bias=bias_sb[:, ot : ot + 1],
                    )
                    nc.sync.dma_start(
                        out_d[ot * 128 : (ot + 1) * 128, t0_prev : t0_prev + tt],
                        out_sb[:],
                    )

            # ---- software-pipelined emission over the flat pair list ----
            # stage A (iteration p): gathers for pair p+GLEAD
            # stage B (iteration p): raw mm + dd + P + S for pair p
            # stage C (iteration p): main matmuls for pair p-LEAD
            # so the PE queue never has main-mms (blocked on S) ahead of the
            # raw mms that unblock the chain.
            LEAD = 2
            GLEAD = 1  # gathers this many pairs ahead of their raw mm
            total = n_chunks * npair
            ps_by_chunk = {}
            S_tiles = {}
            wm_next = 0

            def emit_gathers(p):
                chunk, gp = divmod(p, npair)
                t0 = chunk * tt
                xp = xbpool.tile([128, 2 * XW], BF16, tag="Xp", name=f"xp{p}")
                dxp = dxbpool.tile([128, 2 * DXW], BF16, tag="DXp", name=f"dxp{p}")
                for g2 in range(2):
                    g = 2 * gp + g2
                    nc.gpsimd.dma_start(
                        xp[:, g2 * XW : (g2 + 1) * XW], gather_src(x_d, g, t0, XW)
                    )
                    nc.sync.dma_start(
                        dxp[:, g2 * DXW : (g2 + 1) * DXW],
                        gather_src(dx_d, g, t0, DXW),
                    )
                return xp, dxp

            tiles = {0: emit_gathers(0)}
            for p in range(total + LEAD):
                nonloc_chunk = None
                if p + GLEAD < total and (p + GLEAD) not in tiles:
                    tiles[p + GLEAD] = emit_gathers(p + GLEAD)
                # wm blocks on the scalar HWDGE ring (parallel to sync's rw)
                if wm_next < 4:
                    nc.scalar.dma_start(
                        wm_sb[:, wm_next * 8 * O : (wm_next + 1) * 8 * O],
                        wm_d[:, wm_next * 8 * O : (wm_next + 1) * 8 * O],
                    )
                    wm_next += 1
                if p < total:
                    chunk, gp = divmod(p, npair)
                    xp, dxp = tiles.pop(p)
                    rp = rawps.tile(
                        [128, 2 * tt], F32, tag="rawps", name=f"rp{p}"
                    )
                    ddp = chain.tile([128, 2 * tt], BF16, tag="dd")
                    for g2 in range(2):
                        g = 2 * gp + g2
                        # raw offset-conv matmul: full-128 contraction, banded
                        # weights; rhs col base 5 (even g, taps k'=0..2) or
                        # 1 (odd g, taps k'=4..6) absorbs the tap base.
                        rbase = g2 * XW + (5 if g2 == 0 else 1)
                        nc.tensor.matmul(
                            rp[:, g2 * tt : (g2 + 1) * tt],
                            rw_sb[:, g * 128 : (g + 1) * 128],
                            xp[:, rbase : rbase + tt],
                            start=True, stop=True,
                        )
                        nc.scalar.activation(
                            ddp[:, g2 * tt : (g2 + 1) * tt],
                            rp[:, g2 * tt : (g2 + 1) * tt],
                            Act.Abs, bias=offb_sb[:, g : g + 1],
                        )
                    P = chain.tile([128, 2 * tt], BF16, tag="P")
                    nc.vector.tensor_tensor(P[:], ddp[:], dxp[:], Alu.mult)
                    S = spool.tile([128, 2 * tt], BF16, tag="S", name=f"S{p}")
                    x1 = xp[:]
                    x1.ap = bass_rust.VecI64Pair([(2 * XW, 128), (XW, 2), (1, tt)])
                    x1.offset = 0
                    nc.vector.tensor_tensor(S[:], x1, P[:], Alu.subtract)
                    S_tiles[p] = S
                pm = p - LEAD
                if pm >= 0:
                    chunk, gp = divmod(pm, npair)
                    if chunk not in ps_by_chunk:
                        ps_by_chunk[chunk] = {
                            ot: pspool.tile(
                                [128, tt], F32, tag=f"ps{ot}", name=f"ps{chunk}_{ot}"
                            )
                            for ot in range(n_ot)
                        }
                    ps = ps_by_chunk[chunk]
                    S = S_tiles.pop(pm)
                    for g2 in range(2):
                        g = 2 * gp + g2
                        for ot in range(n_ot):
                            nc.tensor.matmul(
                                ps[ot][:],
                                wm_sb[:, g * O + ot * 128 : g * O + (ot + 1) * 128],
                                S[:, g2 * tt : (g2 + 1) * tt],
                                start=(g == 0),
                                stop=(g == NG - 1),
                            )
                    if gp == npair - 1:
                        # last pair of this chunk: evacuate promptly so the
                        # Scalar queue isn't blocked behind next chunk's dd
                        emit_evac(ps_by_chunk.pop(chunk), chunk * tt)

    nc.compile()
    return nc


def prep_host_inputs(x, offset_w, offset_b, weight, bias, th=TH):
    ow = offset_w.reshape(C, K, OK).astype(np.float32)  # [c, k, j]
    ob = offset_b.reshape(C, K).astype(np.float32)

    # main weight flat image: wm[p=k*16+cl, g*O+o] = weight[o, g*16+cl, k]
    wm = np.ascontiguousarray(
        weight.transpose(1, 2, 0)  # [C, K, O]
        .reshape(NG, 16, K, O)
        .transpose(2, 1, 0, 3)  # [k, cl, g, o]
        .reshape(128, NG * O)
    ).astype(ml_dtypes.bfloat16)

    # raw weight flat image: per g a [128,128] block; even g rows j*16+cl,
    # odd g rows 64+j*16+cl; cols k*16+cl
    rw = np.zeros((128, NG, 128), np.float32)
    cl = np.arange(16)
    for g in range(NG):
        base = 0 if g % 2 == 0 else 64
        for j in range(OK):
            for k in range(K):
                rw[base + j * 16 + cl, g, k * 16 + cl] = ow[g * 16 + cl, k, j]
    rw = np.ascontiguousarray(rw.reshape(128, NG * 128)).astype(ml_dtypes.bfloat16)

    offb = np.ascontiguousarray(
        ob.reshape(NG, 16, K).transpose(2, 1, 0).reshape(128, NG)
    ).astype(np.float32)
    biasr = np.ascontiguousarray(bias.reshape(O // 128, 128).T).astype(np.float32)

    # dx[b, c, v] = x[b, c, v] - x[b, c, v-1]  (x[-1] == 0)
    dxg = np.diff(np.pad(x, ((0, 0), (0, 0), (1, 0))), axis=2)

    xcores, dxcores = [], []
    n_th = T // th
    for core in range(N_CORES):
        b, thi = divmod(core, n_th)
        t0 = thi * th
        xc = np.zeros((C, H + th + HR), np.float32)
        dxc = np.zeros((C, H + th + HR), np.float32)
        xc[:, H : H + th] = x[b, :, t0 : t0 + th]
        dxc[:, H : H + th] = dxg[b, :, t0 : t0 + th]
        if t0 >= H:
            xc[:, :H] = x[b, :, t0 - H : t0]
            dxc[:, :H] = dxg[b, :, t0 - H : t0]
        xcores.append(np.ascontiguousarray(xc.astype(ml_dtypes.bfloat16)))
        dxcores.append(np.ascontiguousarray(dxc.astype(ml_dtypes.bfloat16)))
    return wm, rw, offb, biasr, xcores, dxcores


_PROGRAM_CACHE = {}


def _get_program():
    key = "main"
    if key not in _PROGRAM_CACHE:
        _PROGRAM_CACHE[key] = build_device_program()
    return _PROGRAM_CACHE[key]


def run_on_hw(inputs, trace=False, **kw):
    from concourse.bass_utils import run_bass_kernel_spmd

    nc = _get_program()
    wm, rw, offb, biasr, xcores, dxcores = prep_host_inputs(
        inputs["x"], inputs["offset_w"], inputs["offset_b"],
        inputs["weight"], inputs["bias"],
    )
    in_maps = [
        {
            "xcore": xcores[core],
            "dxcore": dxcores[core],
            "wmain": wm,
            "raww": rw,
            "offb": offb,
            "biasr": biasr,
        }
        for core in range(N_CORES)
    ]
    res = run_bass_kernel_spmd(
        nc, in_maps, core_ids=list(range(N_CORES)), trace=trace, **kw
    )
    return res


def kernel(**inputs) -> np.ndarray:
    res = run_on_hw(inputs)
    out = np.empty((B, O, T), np.float32)
    n_th = T // TH
    for core in range(N_CORES):
        b, thi = divmod(core, n_th)
        out[b, :, thi * TH : (thi + 1) * TH] = res.results[core]["out"].astype(
            np.float32
        )
    return out


if __name__ == "__main__":
    z = np.load("/root/problem/inputs.npz")
    out = kernel(**{k: z[k] for k in z.files})
    print("kernel out:", out.shape, out.dtype, float(np.abs(out).max()))


# revision 9
# speedup vs baseline: 1.2959x; 1.2046x over previous
"""Deformable causal conv1d Trainium2 kernel (v8).

vs v7: sync-overhead-focused rework.
  * Pair-batched gathers: ONE dma_start loads both groups of a pair
    (X: [128, 2*XW], dx: [128, 1024]) -> half the DMA issues + sems.
  * Pair-batched DVE chain: P and S are single [128,1024] TTs; dd is
    two 512-col activations into one shared tile (bias differs per g).
  * Raw matmuls back to full-128 contraction (v6b style, no
    tile_position pairing -- that raced on HW).
  * Weights preloaded as two flat DRAM images ([128, NG*128] raw,
    [128, NG*512] main); rw is loaded FIRST so the PE prologue is
    short (v7 lost 41us waiting for rw behind 4MB of wm loads).
  * Aligned gathers + host-precomputed dx (v7's win, kept): X1 at
    col 0 -> all TTs run in DVE 2x mode.

Sharding: 8 cores = 4 batches x 2 time-halves. No collectives.
"""

import numpy as np
import ml_dtypes
import bass_rust

import concourse.bass as bass
import concourse.tile as tile
from concourse import bacc, mybir

F32 = mybir.dt.float32
BF16 = mybir.dt.bfloat16
Alu = mybir.AluOpType
Act = mybir.ActivationFunctionType

B, C, T = 4, 512, 4096
K, OK = 8, 3
O = 512  # C_out
H = 16  # left halo columns in the x slice
HR = 8  # right pad columns
TH = 2048  # time columns per core
N_CORES = 8
NG = 32  # channel groups of 16
XW = 520  # X gather width per group: X1 at col t, raw rhs at col t+5 / t+1
DXW = 512


def build_device_program(th=TH, tt=512):
    n_chunks = th // tt
    xrow = H + th + HR  # 2072
    npair = NG // 2

    nc = bacc.Bacc("TRN2", target_bir_lowering=False, debug=False)

    x_d = nc.dram_tensor("ximg", [NG // 4 * 128, 4 * xrow], BF16, kind="ExternalInput")
    dx_d = nc.dram_tensor("dximg", [NG // 4 * 128, 4 * xrow], BF16, kind="ExternalInput")
    wm_d = nc.dram_tensor("wmain", [128, NG * O], BF16, kind="ExternalInput").ap()
    rw_d = nc.dram_tensor("raww", [128, NG * 128], BF16, kind="ExternalInput").ap()
    offb_d = nc.dram_tensor("offb", [128, NG], F32, kind="ExternalInput").ap()
    bias_d = nc.dram_tensor("biasr", [128, O // 128], F32, kind="ExternalInput").ap()
    out_d = nc.dram_tensor("out", [O, th], BF16, kind="ExternalOutput").ap()

    n_ot = O // 128

    def quad_src(dram, q, t0, width):
        """Quad image source AP (p:128, g4:4, col:width); element =
        img[q*128 + p, g4*xrow + H + t0 - 7 + col] where img row p=(k,cl)
        already holds x[(4q+g4)*16+cl, u+k]."""
        a = dram.ap()
        a.ap = bass_rust.VecI64Pair([(4 * xrow, 128), (xrow, 4), (1, width)])
        a.offset = q * 128 * 4 * xrow + (H + t0 - 7)
        return a

    def quad_dst(t, width):
        d = t[:]
        d.ap = bass_rust.VecI64Pair([(4 * width, 128), (width, 4), (1, width)])
        d.offset = 0
        return d

    with tile.TileContext(nc) as tc:
        with (
            tc.tile_pool(name="const", bufs=1) as cpool,
            tc.tile_pool(name="xb", bufs=6) as xbpool,
            tc.tile_pool(name="dxb", bufs=6) as dxbpool,
            tc.tile_pool(name="chain", bufs=4) as chain,
            tc.tile_pool(name="spool", bufs=6) as spool,
            tc.tile_pool(name="outp", bufs=2) as outp,
            tc.tile_pool(name="psum", bufs=1, space="PSUM") as pspool,
            tc.tile_pool(name="rawps", bufs=2, space="PSUM") as rawps,
        ):
            # ---- resident constants; rw FIRST so raw matmuls start early ----
            offb_sb = cpool.tile([128, NG], F32, tag="offb")
            nc.sync.dma_start(offb_sb[:], offb_d)
            bias_sb = cpool.tile([128, n_ot], F32, tag="biasr")
            nc.sync.dma_start(bias_sb[:], bias_d)
            rw_sb = cpool.tile([128, NG * 128], BF16, tag="rwall")
            nc.sync.dma_start(rw_sb[:], rw_d)
            wm_sb = cpool.tile([128, NG * O], BF16, tag="wmall")

            def emit_evac(ps_prev, t0_prev):
                for ot in range(n_ot):
                    out_sb = outp.tile([128, tt], BF16, tag="osb", name="out_sb")
                    nc.scalar.activation(
                        out_sb[:], ps_prev[ot][:], Act.Identity,
                        bias=bias_sb[:, ot : ot + 1],
                    )
                    nc.sync.dma_start(
                        out_d[ot * 128 : (ot + 1) * 128, t0_prev : t0_prev + tt],
                        out_sb[:],
                    )

            # ---- software-pipelined emission over the flat pair list ----
            # stage A (iteration p): gathers for pair p+GLEAD
            # stage B (iteration p): raw mm + dd + P + S for pair p
            # stage C (iteration p): main matmuls for pair p-LEAD
            # so the PE queue never has main-mms (blocked on S) ahead of the
            # raw mms that unblock the chain.
            LEAD = 2
            GLEAD = 1  # gathers this many pairs ahead of their raw mm
            total = n_chunks * npair
            ps_by_chunk = {}
            S_tiles = {}
            wm_next = 0

            def emit_gathers(pq):
                # one quad = 2 consecutive pairs (4 groups), single DMA each
                chunk, qp = divmod(pq, npair // 2)
                t0 = chunk * tt
                xq = xbpool.tile([128, 4 * XW], BF16, tag="Xq", name=f"xq{pq}")
                dxq = dxbpool.tile([128, 4 * DXW], BF16, tag="DXq", name=f"dxq{pq}")
                q = chunk * (npair // 2) + qp  # quad index within core run
                nc.gpsimd.dma_start(quad_dst(xq, XW), quad_src(x_d, qp, t0, XW))
                nc.sync.dma_start(quad_dst(dxq, DXW), quad_src(dx_d, qp, t0, DXW))
                return xq, dxq

            qtiles = {0: emit_gathers(0)}
            for p in range(total + LEAD):
                nq = (p + GLEAD + 2) // 2  # quad needed soon
                if nq * 2 < total and nq not in qtiles:
                    qtiles[nq] = emit_gathers(nq)
                # wm blocks on the scalar HWDGE ring (parallel to sync's rw)
                if wm_next < 4:
                    nc.scalar.dma_start(
                        wm_sb[:, wm_next * 8 * O : (wm_next + 1) * 8 * O],
                        wm_d[:, wm_next * 8 * O : (wm_next + 1) * 8 * O],
                    )
                    wm_next += 1
                if p < total:
                    chunk, gp = divmod(p, npair)
                    xq, dxq = qtiles[p // 2]
                    if p % 2 == 1:
                        qtiles.pop(p // 2)
                    po = (p % 2) * 2 * XW
                    dpo = (p % 2) * 2 * DXW
                    rp = rawps.tile(
                        [128, 2 * tt], F32, tag="rawps", name=f"rp{p}"
                    )
                    ddp = chain.tile([128, 2 * tt], BF16, tag="dd")
                    for g2 in range(2):
                        g = 2 * gp + g2
                        # raw offset-conv matmul: full-128 contraction, banded
                        # weights; rhs col base 5 (even g, taps k'=0..2) or
                        # 1 (odd g, taps k'=4..6) absorbs the tap base.
                        rbase = po + g2 * XW + (5 if g2 == 0 else 1)
                        nc.tensor.matmul(
                            rp[:, g2 * tt : (g2 + 1) * tt],
                            rw_sb[:, g * 128 : (g + 1) * 128],
                            xq[:, rbase : rbase + tt],
                            start=True, stop=True,
                        )
                        nc.scalar.activation(
                            ddp[:, g2 * tt : (g2 + 1) * tt],
                            rp[:, g2 * tt : (g2 + 1) * tt],
                            Act.Abs, bias=offb_sb[:, g : g + 1],
                        )
                    P = chain.tile([128, 2 * tt], BF16, tag="P")
                    nc.vector.tensor_tensor(
                        P[:], ddp[:], dxq[:, dpo : dpo + 2 * tt], Alu.mult
                    )
                    S = spool.tile([128, 2 * tt], BF16, tag="S", name=f"S{p}")
                    x1 = xq[:]
                    x1.ap = bass_rust.VecI64Pair([(4 * XW, 128), (XW, 2), (1, tt)])
                    x1.offset = po
                    nc.vector.tensor_tensor(S[:], x1, P[:], Alu.subtract)
                    S_tiles[p] = S
                pm = p - LEAD
                if pm >= 0:
                    chunk, gp = divmod(pm, npair)
                    if chunk not in ps_by_chunk:
                        ps_by_chunk[chunk] = {
                            ot: pspool.tile(
                                [128, tt], F32, tag=f"ps{ot}", name=f"ps{chunk}_{ot}"
                            )
                            for ot in range(n_ot)
                        }
                    ps = ps_by_chunk[chunk]
                    S = S_tiles.pop(pm)
                    for g2 in range(2):
                        g = 2 * gp + g2
                        for ot in range(n_ot):
                            nc.tensor.matmul(
                                ps[ot][:],
                                wm_sb[:, g * O + ot * 128 : g * O + (ot + 1) * 128],
                                S[:, g2 * tt : (g2 + 1) * tt],
                                start=(g == 0),
                                stop=(g == NG - 1),
                            )
                    if gp == npair - 1:
                        # last pair of this chunk: evacuate promptly so the
                        # Scalar queue isn't blocked behind next chunk's dd
                        emit_evac(ps_by_chunk.pop(chunk), chunk * tt)

    nc.compile()
    return nc


def prep_host_inputs(x, offset_w, offset_b, weight, bias, th=TH):
    ow = offset_w.reshape(C, K, OK).astype(np.float32)  # [c, k, j]
    ob = offset_b.reshape(C, K).astype(np.float32)

    # main weight flat image: wm[p=k*16+cl, g*O+o] = weight[o, g*16+cl, k]
    wm = np.ascontiguousarray(
        weight.transpose(1, 2, 0)  # [C, K, O]
        .reshape(NG, 16, K, O)
        .transpose(2, 1, 0, 3)  # [k, cl, g, o]
        .reshape(128, NG * O)
    ).astype(ml_dtypes.bfloat16)

    # raw weight flat image: per g a [128,128] block; even g rows j*16+cl,
    # odd g rows 64+j*16+cl; cols k*16+cl
    rw = np.zeros((128, NG, 128), np.float32)
    cl = np.arange(16)
    for g in range(NG):
        base = 0 if g % 2 == 0 else 64
        for j in range(OK):
            for k in range(K):
                rw[base + j * 16 + cl, g, k * 16 + cl] = ow[g * 16 + cl, k, j]
    rw = np.ascontiguousarray(rw.reshape(128, NG * 128)).astype(ml_dtypes.bfloat16)

    offb = np.ascontiguousarray(
        ob.reshape(NG, 16, K).transpose(2, 1, 0).reshape(128, NG)
    ).astype(np.float32)
    biasr = np.ascontiguousarray(bias.reshape(O // 128, 128).T).astype(np.float32)

    # dx[b, c, v] = x[b, c, v] - x[b, c, v-1]  (x[-1] == 0)
    dxg = np.diff(np.pad(x, ((0, 0), (0, 0), (1, 0))), axis=2)

    def quad_image(arr):
        # arr [C, xrow] -> img [8*128, 4*xrow]:
        # img[q*128 + k*16+cl, g4*xrow + u] = arr[(4q+g4)*16+cl, u+k]
        xrow = arr.shape[1]
        img = np.zeros((8, 8, 16, 4, xrow), arr.dtype)  # [q, k, cl, g4, u]
        a = arr.reshape(8, 4, 16, xrow)  # [q, g4, cl, u]
        for k in range(K):
            img[:, k, :, :, : xrow - k] = a[:, :, :, k:].transpose(0, 2, 1, 3)
        return np.ascontiguousarray(
            img.transpose(0, 1, 2, 3, 4).reshape(8 * 128, 4 * xrow)
        )

    xcores, dxcores = [], []
    n_th = T // th
    for core in range(N_CORES):
        b, thi = divmod(core, n_th)
        t0 = thi * th
        xc = np.zeros((C, H + th + HR), np.float32)
        dxc = np.zeros((C, H + th + HR), np.float32)
        xc[:, H : H + th] = x[b, :, t0 : t0 + th]
        dxc[:, H : H + th] = dxg[b, :, t0 : t0 + th]
        if t0 >= H:
            xc[:, :H] = x[b, :, t0 - H : t0]
            dxc[:, :H] = dxg[b, :, t0 - H : t0]
        xcores.append(quad_image(xc.astype(ml_dtypes.bfloat16)))
        dxcores.append(quad_image(dxc.astype(ml_dtypes.bfloat16)))
    return wm, rw, offb, biasr, xcores, dxcores


_PROGRAM_CACHE = {}


def _get_program():
    key = "main"
    if key not in _PROGRAM_CACHE:
        _PROGRAM_CACHE[key] = build_device_program()
    return _PROGRAM_CACHE[key]


def run_on_hw(inputs, trace=False, **kw):
    from concourse.bass_utils import run_bass_kernel_spmd

    nc = _get_program()
    wm, rw, offb, biasr, xcores, dxcores = prep_host_inputs(
        inputs["x"], inputs["offset_w"], inputs["offset_b"],
        inputs["weight"], inputs["bias"],
    )
    in_maps = [
        {
            "ximg": xcores[core],
            "dximg": dxcores[core],
            "wmain": wm,
            "raww": rw,
            "offb": offb,
            "biasr": biasr,
        }
        for core in range(N_CORES)
    ]
    res = run_bass_kernel_spmd(
        nc, in_maps, core_ids=list(range(N_CORES)), trace=trace, **kw
    )
    return res


def kernel(**inputs) -> np.ndarray:
    res = run_on_hw(inputs)
    out = np.empty((B, O, T), np.float32)
    n_th = T // TH
    for core in range(N_CORES):
        b, thi = divmod(core, n_th)
        out[b, :, thi * TH : (thi + 1) * TH] = res.results[core]["out"].astype(
            np.float32
        )
    return out


if __name__ == "__main__":
    z = np.load("/root/problem/inputs.npz")
    out = kernel(**{k: z[k] for k in z.files})
    print("kernel out:", out.shape, out.dtype, float(np.abs(out).max()))


# revision 11
# speedup vs baseline: 1.3311x; 1.0272x over previous
"""Deformable causal conv1d Trainium2 kernel (v8).

vs v7: sync-overhead-focused rework.
  * Pair-batched gathers: ONE dma_start loads both groups of a pair
    (X: [128, 2*XW], dx: [128, 1024]) -> half the DMA issues + sems.
  * Pair-batched DVE chain: P and S are single [128,1024] TTs; dd is
    two 512-col activations into one shared tile (bias differs per g).
  * Raw matmuls back to full-128 contraction (v6b style, no
    tile_position pairing -- that raced on HW).
  * Weights preloaded as two flat DRAM images ([128, NG*128] raw,
    [128, NG*512] main); rw is loaded FIRST so the PE prologue is
    short (v7 lost 41us waiting for rw behind 4MB of wm loads).
  * Aligned gathers + host-precomputed dx (v7's win, kept): X1 at
    col 0 -> all TTs run in DVE 2x mode.

Sharding: 8 cores = 4 batches x 2 time-halves. No collectives.
"""

import numpy as np
import ml_dtypes
import bass_rust

import concourse.bass as bass
import concourse.tile as tile
from concourse import bacc, mybir

F32 = mybir.dt.float32
BF16 = mybir.dt.bfloat16
Alu = mybir.AluOpType
Act = mybir.ActivationFunctionType

B, C, T = 4, 512, 4096
K, OK = 8, 3
O = 512  # C_out
H = 16  # left halo columns in the x slice
HR = 8  # right pad columns
TH = 2048  # time columns per core
N_CORES = 8
NG = 32  # channel groups of 16
XW = 520  # X gather width per group: X1 at col t, raw rhs at col t+5 / t+1
DXW = 512


def build_device_program(th=TH, tt=512):
    n_chunks = th // tt
    xrow = H + th + HR  # 2072
    npair = NG // 2

    nc = bacc.Bacc("TRN2", target_bir_lowering=False, debug=False)

    x_d = nc.dram_tensor("ximg", [NG // 4 * 128, 4 * xrow], BF16, kind="ExternalInput")
    dx_d = nc.dram_tensor("dximg", [NG // 4 * 128, 4 * xrow], BF16, kind="ExternalInput")
    wm_d = nc.dram_tensor("wmain", [128, NG * O], BF16, kind="ExternalInput").ap()
    rw_d = nc.dram_tensor("raww", [128, NG * 128], BF16, kind="ExternalInput").ap()
    offb_d = nc.dram_tensor("offb", [128, NG], F32, kind="ExternalInput").ap()
    bias_d = nc.dram_tensor("biasr", [128, O // 128], F32, kind="ExternalInput").ap()
    out_d = nc.dram_tensor("out", [O, th], BF16, kind="ExternalOutput").ap()

    n_ot = O // 128

    def quad_src(dram, q, t0, width):
        """Quad image source AP (p:128, g4:4, col:width); element =
        img[q*128 + p, g4*xrow + H + t0 - 7 + col] where img row p=(k,cl)
        already holds x[(4q+g4)*16+cl, u+k]."""
        a = dram.ap()
        a.ap = bass_rust.VecI64Pair([(4 * xrow, 128), (xrow, 4), (1, width)])
        a.offset = q * 128 * 4 * xrow + (H + t0 - 7)
        return a

    def quad_dst(t, width):
        d = t[:]
        d.ap = bass_rust.VecI64Pair([(4 * width, 128), (width, 4), (1, width)])
        d.offset = 0
        return d

    with tile.TileContext(nc) as tc:
        with (
            tc.tile_pool(name="const", bufs=1) as cpool,
            tc.tile_pool(name="xb", bufs=6) as xbpool,
            tc.tile_pool(name="dxb", bufs=6) as dxbpool,
            tc.tile_pool(name="chain", bufs=4) as chain,
            tc.tile_pool(name="spool", bufs=6) as spool,
            tc.tile_pool(name="outp", bufs=2) as outp,
            tc.tile_pool(name="psum", bufs=1, space="PSUM") as pspool,
            tc.tile_pool(name="rawps", bufs=2, space="PSUM") as rawps,
        ):
            # ---- resident constants; rw FIRST so raw matmuls start early ----
            rw_sb = cpool.tile([128, NG * 128], BF16, tag="rwall")
            nc.sync.dma_start(rw_sb[:, : 16 * 128], rw_d[:, : 16 * 128])
            offb_sb = cpool.tile([128, NG], F32, tag="offb")
            nc.sync.dma_start(offb_sb[:], offb_d)
            bias_sb = cpool.tile([128, n_ot], F32, tag="biasr")
            nc.sync.dma_start(bias_sb[:], bias_d)
            nc.sync.dma_start(rw_sb[:, 16 * 128 :], rw_d[:, 16 * 128 :])
            wm_sb = cpool.tile([128, NG * O], BF16, tag="wmall")

            def emit_evac(ps_prev, t0_prev):
                for ot in range(n_ot):
                    out_sb = outp.tile([128, tt], BF16, tag="osb", name="out_sb")
                    nc.scalar.activation(
                        out_sb[:], ps_prev[ot][:], Act.Identity,
                        bias=bias_sb[:, ot : ot + 1],
                    )
                    nc.sync.dma_start(
                        out_d[ot * 128 : (ot + 1) * 128, t0_prev : t0_prev + tt],
                        out_sb[:],
                    )

            # ---- software-pipelined emission over the flat pair list ----
            # stage A (iteration p): gathers for pair p+GLEAD
            # stage B (iteration p): raw mm + dd + P + S for pair p
            # stage C (iteration p): main matmuls for pair p-LEAD
            # so the PE queue never has main-mms (blocked on S) ahead of the
            # raw mms that unblock the chain.
            LEAD = 2
            GLEAD = 1  # gathers this many pairs ahead of their raw mm
            total = n_chunks * npair
            ps_by_chunk = {}
            S_tiles = {}
            wm_next = 0

            def emit_gathers(pq):
                # one quad = 2 consecutive pairs (4 groups), single DMA each
                chunk, qp = divmod(pq, npair // 2)
                t0 = chunk * tt
                xq = xbpool.tile([128, 4 * XW], BF16, tag="Xq", name=f"xq{pq}")
                dxq = dxbpool.tile([128, 4 * DXW], BF16, tag="DXq", name=f"dxq{pq}")
                q = chunk * (npair // 2) + qp  # quad index within core run
                nc.sync.dma_start(quad_dst(xq, XW), quad_src(x_d, qp, t0, XW))
                nc.sync.dma_start(quad_dst(dxq, DXW), quad_src(dx_d, qp, t0, DXW))
                return xq, dxq

            qtiles = {0: emit_gathers(0)}
            q_emitted = 0
            for p in range(total + LEAD):
                nq = (p + GLEAD + 4) // 2  # keep gathers ~2.5 quads ahead
                while q_emitted < nq and (q_emitted + 1) * 2 < total:
                    q_emitted += 1
                    qtiles[q_emitted] = emit_gathers(q_emitted)
                # wm blocks on the scalar HWDGE ring (parallel to sync's rw)
                if wm_next < 4:
                    nc.scalar.dma_start(
                        wm_sb[:, wm_next * 8 * O : (wm_next + 1) * 8 * O],
                        wm_d[:, wm_next * 8 * O : (wm_next + 1) * 8 * O],
                    )
                    wm_next += 1
                if p < total:
                    chunk, gp = divmod(p, npair)
                    xq, dxq = qtiles[p // 2]
                    if p % 2 == 1:
                        qtiles.pop(p // 2)
                    po = (p % 2) * 2 * XW
                    dpo = (p % 2) * 2 * DXW
                    rp = rawps.tile(
                        [128, 2 * tt], F32, tag="rawps", name=f"rp{p}"
                    )
                    ddp = chain.tile([128, 2 * tt], BF16, tag="dd")
                    for g2 in range(2):
                        g = 2 * gp + g2
                        # raw offset-conv matmul: full-128 contraction, banded
                        # weights; rhs col base 5 (even g, taps k'=0..2) or
                        # 1 (odd g, taps k'=4..6) absorbs the tap base.
                        rbase = po + g2 * XW + (5 if g2 == 0 else 1)
                        nc.tensor.matmul(
                            rp[:, g2 * tt : (g2 + 1) * tt],
                            rw_sb[:, g * 128 : (g + 1) * 128],
                            xq[:, rbase : rbase + tt],
                            start=True, stop=True,
                        )
                        nc.scalar.activation(
                            ddp[:, g2 * tt : (g2 + 1) * tt],
                            rp[:, g2 * tt : (g2 + 1) * tt],
                            Act.Abs, bias=offb_sb[:, g : g + 1],
                        )
                    P = chain.tile([128, 2 * tt], BF16, tag="P")
                    nc.vector.tensor_tensor(
                        P[:], ddp[:], dxq[:, dpo : dpo + 2 * tt], Alu.mult
                    )
                    S = spool.tile([128, 2 * tt], BF16, tag="S", name=f"S{p}")
                    x1 = xq[:]
                    x1.ap = bass_rust.VecI64Pair([(4 * XW, 128), (XW, 2), (1, tt)])
                    x1.offset = po
                    nc.vector.tensor_tensor(S[:], x1, P[:], Alu.subtract)
                    S_tiles[p] = S
                pm = p - LEAD
                if pm >= 0:
                    chunk, gp = divmod(pm, npair)
                    if chunk not in ps_by_chunk:
                        ps_by_chunk[chunk] = {
                            ot: pspool.tile(
                                [128, tt], F32, tag=f"ps{ot}", name=f"ps{chunk}_{ot}"
                            )
                            for ot in range(n_ot)
                        }
                    ps = ps_by_chunk[chunk]
                    S = S_tiles.pop(pm)
                    for g2 in range(2):
                        g = 2 * gp + g2
                        for ot in range(n_ot):
                            nc.tensor.matmul(
                                ps[ot][:],
                                wm_sb[:, g * O + ot * 128 : g * O + (ot + 1) * 128],
                                S[:, g2 * tt : (g2 + 1) * tt],
                                start=(g == 0),
                                stop=(g == NG - 1),
                            )
                    if gp == npair - 1:
                        # last pair of this chunk: evacuate promptly so the
                        # Scalar queue isn't blocked behind next chunk's dd
                        emit_evac(ps_by_chunk.pop(chunk), chunk * tt)

    nc.compile()
    return nc


def prep_host_inputs(x, offset_w, offset_b, weight, bias, th=TH):
    ow = offset_w.reshape(C, K, OK).astype(np.float32)  # [c, k, j]
    ob = offset_b.reshape(C, K).astype(np.float32)

    # main weight flat image: wm[p=k*16+cl, g*O+o] = weight[o, g*16+cl, k]
    wm = np.ascontiguousarray(
        weight.transpose(1, 2, 0)  # [C, K, O]
        .reshape(NG, 16, K, O)
        .transpose(2, 1, 0, 3)  # [k, cl, g, o]
        .reshape(128, NG * O)
    ).astype(ml_dtypes.bfloat16)

    # raw weight flat image: per g a [128,128] block; even g rows j*16+cl,
    # odd g rows 64+j*16+cl; cols k*16+cl
    rw = np.zeros((128, NG, 128), np.float32)
    cl = np.arange(16)
    for g in range(NG):
        base = 0 if g % 2 == 0 else 64
        for j in range(OK):
            for k in range(K):
                rw[base + j * 16 + cl, g, k * 16 + cl] = ow[g * 16 + cl, k, j]
    rw = np.ascontiguousarray(rw.reshape(128, NG * 128)).astype(ml_dtypes.bfloat16)

    offb = np.ascontiguousarray(
        ob.reshape(NG, 16, K).transpose(2, 1, 0).reshape(128, NG)
    ).astype(np.float32)
    biasr = np.ascontiguousarray(bias.reshape(O // 128, 128).T).astype(np.float32)

    # dx[b, c, v] = x[b, c, v] - x[b, c, v-1]  (x[-1] == 0)
    dxg = np.diff(np.pad(x, ((0, 0), (0, 0), (1, 0))), axis=2)

    def quad_image(arr):
        # arr [C, xrow] -> img [8*128, 4*xrow]:
        # img[q*128 + k*16+cl, g4*xrow + u] = arr[(4q+g4)*16+cl, u+k]
        xrow = arr.shape[1]
        img = np.zeros((8, 8, 16, 4, xrow), arr.dtype)  # [q, k, cl, g4, u]
        a = arr.reshape(8, 4, 16, xrow)  # [q, g4, cl, u]
        for k in range(K):
            img[:, k, :, :, : xrow - k] = a[:, :, :, k:].transpose(0, 2, 1, 3)
        return np.ascontiguousarray(
            img.transpose(0, 1, 2, 3, 4).reshape(8 * 128, 4 * xrow)
        )

    xcores, dxcores = [], []
    n_th = T // th
    for core in range(N_CORES):
        b, thi = divmod(core, n_th)
        t0 = thi * th
        xc = np.zeros((C, H + th + HR), np.float32)
        dxc = np.zeros((C, H + th + HR), np.float32)
        xc[:, H : H + th] = x[b, :, t0 : t0 + th]
        dxc[:, H : H + th] = dxg[b, :, t0 : t0 + th]
        if t0 >= H:
            xc[:, :H] = x[b, :, t0 - H : t0]
            dxc[:, :H] = dxg[b, :, t0 - H : t0]
        xcores.append(quad_image(xc.astype(ml_dtypes.bfloat16)))
        dxcores.append(quad_image(dxc.astype(ml_dtypes.bfloat16)))
    return wm, rw, offb, biasr, xcores, dxcores


_PROGRAM_CACHE = {}


def _get_program():
    key = "main"
    if key not in _PROGRAM_CACHE:
        _PROGRAM_CACHE[key] = build_device_program()
    return _PROGRAM_CACHE[key]


def run_on_hw(inputs, trace=False, **kw):
    from concourse.bass_utils import run_bass_kernel_spmd

    nc = _get_program()
    wm, rw, offb, biasr, xcores, dxcores = prep_host_inputs(
        inputs["x"], inputs["offset_w"], inputs["offset_b"],
        inputs["weight"], inputs["bias"],
    )
    in_maps = [
        {
            "ximg": xcores[core],
            "dximg": dxcores[core],
            "wmain": wm,
            "raww": rw,
            "offb": offb,
            "biasr": biasr,
        }
        for core in range(N_CORES)
    ]
    res = run_bass_kernel_spmd(
        nc, in_maps, core_ids=list(range(N_CORES)), trace=trace, **kw
    )
    return res


def kernel(**inputs) -> np.ndarray:
    res = run_on_hw(inputs)
    out = np.empty((B, O, T), np.float32)
    n_th = T // TH
    for core in range(N_CORES):
        b, thi = divmod(core, n_th)
        out[b, :, thi * TH : (thi + 1) * TH] = res.results[core]["out"].astype(
            np.float32
        )
    return out


if __name__ == "__main__":
    z = np.load("/root/problem/inputs.npz")
    out = kernel(**{k: z[k] for k in z.files})
    print("kernel out:", out.shape, out.dtype, float(np.abs(out).max()))
